# revision 1
# baseline (speedup 1.0000x reference)
"""CorrelationSampler Trainium2 kernel.

out[b, h, w, c] = bilinear sample of corr[b, :, :, c] at grid position
(h + flow_y, w + flow_x)-ish (align_corners=True, border padding).

Strategy:
  - Host computes integer corner indices and the 4 bilinear weights per
    output position (cheap: B*H*W = 16K positions).
  - Corner indices are re-clamped so ix1 == ix0+1 always (ix0 <= W-2),
    which is mathematically identical to the reference clipping and makes
    the two x-neighbors one contiguous 2*4096-float chunk in memory.
  - 8 cores = batch (4) x position-half (2). Each core gathers row-pairs
    of its batch's [4096, 4096] correlation matrix with indirect DMA and
    blends them on the vector engine with per-partition scalar weights.
"""

import numpy as np

B, H, W = 4, 64, 64
HW = H * W  # 4096 channels; also 4096 source rows per batch
N_CORES = 8
POS_PER_CORE = (B * HW) // N_CORES  # 2048
P = 128  # partitions
N_TILES = POS_PER_CORE // P  # 16


def _host_indices_weights(flow: np.ndarray):
    """float32 replica of the reference's grid math -> corner row indices
    and bilinear corner weights, shape [B, H*W] each."""
    f32 = np.float32
    y_g, x_g = np.meshgrid(
        np.arange(H, dtype=f32), np.arange(W, dtype=f32), indexing="ij"
    )
    x_norm = (f32(2.0) * x_g / f32(W - 1) - f32(1.0)).astype(f32)
    y_norm = (f32(2.0) * y_g / f32(H - 1) - f32(1.0)).astype(f32)

    fx = flow[:, 0].astype(f32)
    fy = flow[:, 1].astype(f32)
    gx = x_norm[None] + fx / f32(W) * f32(2.0)
    gy = y_norm[None] + fy / f32(H) * f32(2.0)

    ix = np.clip((gx + f32(1.0)) * f32(0.5) * f32(W - 1), f32(0.0), f32(W - 1))
    iy = np.clip((gy + f32(1.0)) * f32(0.5) * f32(H - 1), f32(0.0), f32(H - 1))

    # floor is >= 0 after the clip; clamp to W-2/H-2 so the +1 neighbor
    # always exists. At the high border this gives weight 1.0 on the last
    # row/col -- identical result to the reference's clip formulation.
    ix0 = np.minimum(np.floor(ix), f32(W - 2)).astype(np.int32)
    iy0 = np.minimum(np.floor(iy), f32(H - 2)).astype(np.int32)
    wx = (ix - ix0.astype(f32)).astype(f32)
    wy = (iy - iy0.astype(f32)).astype(f32)

    one = f32(1.0)
    w00 = ((one - wy) * (one - wx)).astype(f32)
    w01 = ((one - wy) * wx).astype(f32)
    w10 = (wy * (one - wx)).astype(f32)
    w11 = (wy * wx).astype(f32)

    row0 = iy0 * np.int32(W) + ix0  # gather start row for (iy0, ix0..ix0+1)
    row1 = row0 + np.int32(W)  # (iy0+1, ix0..ix0+1)

    flat = lambda a: a.reshape(B, HW)
    return (
        flat(row0),
        flat(row1),
        flat(w00),
        flat(w01),
        flat(w10),
        flat(w11),
    )


def _build_program():
    import concourse.bacc as bacc
    import concourse.bass as bass
    import concourse.mybir as mybir
    from concourse.tile import TileContext

    f32 = mybir.dt.float32
    i32 = mybir.dt.int32

    nc = bacc.Bacc(
        "TRN2", target_bir_lowering=False, debug=False, num_devices=N_CORES
    )
    corr = nc.dram_tensor("corr", [HW, HW], f32, kind="ExternalInput").ap()
    idx = nc.dram_tensor("idx", [P, 2 * N_TILES + 1], i32, kind="ExternalInput").ap()
    wts = nc.dram_tensor("wts", [P, 4 * N_TILES], f32, kind="ExternalInput").ap()
    out = nc.dram_tensor(
        "out", [POS_PER_CORE, HW], f32, kind="ExternalOutput"
    ).ap()

    mult = mybir.AluOpType.mult
    add = mybir.AluOpType.add

    with TileContext(nc) as tc:
        with (
            tc.tile_pool(name="meta", bufs=1) as meta,
            tc.tile_pool(name="pairs", bufs=2) as pairp,
            tc.tile_pool(name="acc", bufs=4) as accp,
        ):
            idx_t = meta.tile([P, 2 * N_TILES + 1], i32)
            wts_t = meta.tile([P, 4 * N_TILES], f32)
            # idx via gpsimd: same engine as the gathers, avoids a
            # cross-engine semaphore hop on the critical startup path
            nc.gpsimd.dma_start(out=idx_t[:], in_=idx[:])
            nc.sync.dma_start(out=wts_t[:], in_=wts[:])

            for t in range(N_TILES):
                # Two indirect gathers per tile (one per y-row): each
                # partition reads 8192 contiguous floats = source rows
                # (y, x0) and (y, x0+1) -> pair[p] = [a | b] slabs.
                pair0 = pairp.tile([P, 2 * HW], f32, tag="pair0")
                pair1 = pairp.tile([P, 2 * HW], f32, tag="pair1")
                nc.gpsimd.indirect_dma_start(
                    out=pair0[:],
                    out_offset=None,
                    in_=corr[:],
                    in_offset=bass.IndirectOffsetOnAxis(
                        ap=idx_t[:, 2 * t : 2 * t + 1], axis=0
                    ),
                )
                if t < N_TILES - 1:
                    nc.gpsimd.indirect_dma_start(
                        out=pair1[:],
                        out_offset=None,
                        in_=corr[:],
                        in_offset=bass.IndirectOffsetOnAxis(
                            ap=idx_t[:, 2 * t + 1 : 2 * t + 2], axis=0
                        ),
                    )
                else:
                    # split the kernel's final gather: row1 then row1+1,
                    # so the last HBM dependency is half-size and the
                    # closing blend+store starts sooner
                    nc.gpsimd.indirect_dma_start(
                        out=pair1[:, 0:HW],
                        out_offset=None,
                        in_=corr[:],
                        in_offset=bass.IndirectOffsetOnAxis(
                            ap=idx_t[:, 2 * t + 1 : 2 * t + 2], axis=0
                        ),
                    )
                    nc.gpsimd.indirect_dma_start(
                        out=pair1[:, HW : 2 * HW],
                        out_offset=None,
                        in_=corr[:],
                        in_offset=bass.IndirectOffsetOnAxis(
                            ap=idx_t[:, 2 * N_TILES : 2 * N_TILES + 1], axis=0
                        ),
                    )
                w = [wts_t[:, k * N_TILES + t : k * N_TILES + t + 1] for k in range(4)]
                slabs = [pair0, pair0, pair1, pair1]
                # Last tile: blend+store in two channel chunks so the final
                # store overlaps the final blend (shorter pipeline drain).
                n_chunks = 2 if t == N_TILES - 1 else 1
                csz = HW // n_chunks
                for c0 in range(0, HW, csz):
                    acc = accp.tile([P, csz], f32, tag="acc")
                    sl = lambda k: slabs[k][:, (k % 2) * HW + c0 : (k % 2) * HW + c0 + csz]
                    # acc = w00*a + w01*b + w10*c + w11*d
                    nc.vector.tensor_scalar_mul(acc[:], sl(0), w[0])
                    for k in range(1, 4):
                        nc.vector.scalar_tensor_tensor(
                            acc[:], sl(k), w[k], acc[:], mult, add
                        )
                    nc.sync.dma_start(
                        out=out[t * P : (t + 1) * P, c0 : c0 + csz], in_=acc[:]
                    )
    nc.compile()
    return nc


def _core_meta(row0, row1, w00, w01, w10, w11, b, half):
    """Pack per-core idx [P, 2*N_TILES] and wts [P, 4*N_TILES] tensors.

    Core (b, half) handles flat positions [half*2048, (half+1)*2048) of
    batch b. Positions are sorted by gather address (row0) before being
    assigned to (tile, partition) slots: consecutive descriptors then hit
    adjacent/duplicate source rows, which raises the DRAM row-buffer hit
    rate of the random gather stream. The device writes results in sorted
    order; `perm` lets the host scatter rows back at unshard time."""
    sl = slice(half * POS_PER_CORE, (half + 1) * POS_PER_CORE)
    perm = np.argsort(row0[b, sl], kind="stable")
    # [POS_PER_CORE] sorted -> [N_TILES, P] -> [P, N_TILES]
    tp = lambda a: np.ascontiguousarray(a[b, sl][perm].reshape(N_TILES, P).T)
    # idx columns interleaved (row0_t, row1_t) so tile t's offset AP is
    # the [P, 2] slice idx[:, 2t:2t+2]
    idx = np.empty((P, 2 * N_TILES + 1), dtype=np.int32)
    r1 = tp(row1)
    idx[:, 0:-1:2] = tp(row0)
    idx[:, 1:-1:2] = r1
    idx[:, -1] = r1[:, -1] + 1  # last tile's row1+1 for the split gather
    wts = np.concatenate(
        [tp(w00), tp(w01), tp(w10), tp(w11)], axis=1
    ).astype(np.float32)
    return np.ascontiguousarray(idx), np.ascontiguousarray(wts), perm


_cached = {}


def _get_program():
    if "nc" not in _cached:
        _cached["nc"] = _build_program()
    return _cached["nc"]


def _ensure_axon_hooks_importable():
    """bass_utils imports antenv.axon_hooks when tracing is requested (e.g.
    BASS_TRACE=1). Some containers ship an antenv stub without that module;
    provide a no-op registry so tracing degrades gracefully instead of
    crashing the run."""
    import sys
    import types

    try:
        import antenv.axon_hooks  # noqa: F401
    except Exception:
        m = types.ModuleType("antenv.axon_hooks")
        m._hook = None
        m.set_axon_ntff_profile_hook = lambda h: setattr(m, "_hook", h)
        m.get_axon_ntff_profile_hook = lambda: getattr(m, "_hook", None)
        sys.modules["antenv.axon_hooks"] = m


def kernel(correlation: np.ndarray, flow: np.ndarray, _trace: bool = False):
    _ensure_axon_hooks_importable()
    from concourse.bass_utils import run_bass_kernel_spmd

    correlation = np.ascontiguousarray(correlation, dtype=np.float32)
    flow = np.asarray(flow, dtype=np.float32)

    row0, row1, w00, w01, w10, w11 = _host_indices_weights(flow)

    in_maps = []
    perms = []
    for core in range(N_CORES):
        b, half = divmod(core, 2)
        idx, wts, perm = _core_meta(row0, row1, w00, w01, w10, w11, b, half)
        perms.append(perm)
        in_maps.append(
            {
                "corr": correlation[b].reshape(HW, HW),
                "idx": idx,
                "wts": wts,
            }
        )

    nc = _get_program()
    extra = {"trace_cores": list(range(N_CORES))} if _trace else {}
    res = run_bass_kernel_spmd(
        nc, in_maps, core_ids=list(range(N_CORES)), trace=_trace, **extra
    )

    out = np.empty((B, HW, HW), dtype=np.float32)
    for core in range(N_CORES):
        b, half = divmod(core, 2)
        # device rows are in address-sorted order; scatter back to
        # natural position order
        out[b, half * POS_PER_CORE + perms[core], :] = res.results[core]["out"]
    if _trace:
        kernel.last_results = res
    return out.reshape(B, H, W, HW)



# revision 2
# speedup vs baseline: 3.7598x; 3.7598x over previous
"""CorrelationSampler Trainium2 kernel — banded-matmul formulation.

out[b, h, w, c] = bilinear sample of corr[b, :, :, c] at grid position
(h + flow_y, w + flow_x)-ish (align_corners=True, border padding).

Per batch this is out = S @ C where C = corr[b] viewed as a
[4096 src-position, 4096 channel] matrix and S is a [4096, 4096] sparse
matrix with the 4 bilinear corner weights per row. Sorting output
positions by their gather address makes S block-banded: each 128-row
out-block only touches src rows in a 3-aligned-block window around the
diagonal. The tensor engine then does the gather+blend as dense fp16
matmuls while C streams into SBUF exactly once — no 4x gather read
amplification, and fp16 halves all HBM traffic:

  - 8 cores = batch (4) x channel-half (2). Each core computes the full
    4096 sorted positions for its 2048-channel half of C.
  - Per out-block i (128 sorted positions): 3 window blocks x 4
    PSUM-bank matmuls of [128src,128pos]^T @ [128src,512ch], fp32 PSUM
    accumulate, then a PSUM->SBUF fp16 cast copy (alternating DVE /
    Activation engines) and a DMA store.
  - Host does the cheap parts: grid math, position sort, building the
    per-block stationary S^T tiles, fp16 conversion, and the final
    un-permute + fp32 upcast.
"""

import numpy as np

B, H, W = 4, 64, 64
HW = H * W  # 4096 source rows per batch; also 4096 output positions
CH_HALF = HW // 2  # 2048 channels per core
P = 128
N_BLOCKS = HW // P  # 32 out-blocks
KWIN = 3  # aligned src-blocks per out-block window
N_CORES = 8

# static window base per out-block: blocks {w_i, w_i+1, w_i+2}
_WBASE = np.minimum(np.maximum(np.arange(N_BLOCKS) - 1, 0), N_BLOCKS - KWIN)


def _host_indices_weights(flow: np.ndarray):
    """float32 replica of the reference's grid math -> corner row indices
    and bilinear corner weights, shape [B, H*W] each."""
    f32 = np.float32
    y_g, x_g = np.meshgrid(
        np.arange(H, dtype=f32), np.arange(W, dtype=f32), indexing="ij"
    )
    x_norm = (f32(2.0) * x_g / f32(W - 1) - f32(1.0)).astype(f32)
    y_norm = (f32(2.0) * y_g / f32(H - 1) - f32(1.0)).astype(f32)

    fx = flow[:, 0].astype(f32)
    fy = flow[:, 1].astype(f32)
    gx = x_norm[None] + fx / f32(W) * f32(2.0)
    gy = y_norm[None] + fy / f32(H) * f32(2.0)

    ix = np.clip((gx + f32(1.0)) * f32(0.5) * f32(W - 1), f32(0.0), f32(W - 1))
    iy = np.clip((gy + f32(1.0)) * f32(0.5) * f32(H - 1), f32(0.0), f32(H - 1))

    # floor is >= 0 after the clip; clamp to W-2/H-2 so the +1 neighbor
    # always exists. At the high border this gives weight 1.0 on the last
    # row/col -- identical result to the reference's clip formulation.
    ix0 = np.minimum(np.floor(ix), f32(W - 2)).astype(np.int32)
    iy0 = np.minimum(np.floor(iy), f32(H - 2)).astype(np.int32)
    wx = (ix - ix0.astype(f32)).astype(f32)
    wy = (iy - iy0.astype(f32)).astype(f32)

    one = f32(1.0)
    w00 = ((one - wy) * (one - wx)).astype(f32)
    w01 = ((one - wy) * wx).astype(f32)
    w10 = (wy * (one - wx)).astype(f32)
    w11 = (wy * wx).astype(f32)

    r0 = (iy0 * np.int32(W) + ix0).reshape(B, HW)
    flat = lambda a: a.reshape(B, HW)
    return r0, flat(w00), flat(w01), flat(w10), flat(w11)


def _batch_schedule(r0, w00, w01, w10, w11):
    """For one batch: sort positions by gather address, build the
    stationary S^T tiles for the static 3-block windows.

    Returns (perm [HW], st [P, N_BLOCKS*KWIN*P] fp16). Row rk of the
    device output corresponds to original position perm[rk]."""
    perm = np.argsort(r0, kind="stable")
    rk = np.arange(HW)
    blk = rk // P
    lane = rk % P
    st = np.zeros((P, N_BLOCKS * KWIN * P), np.float32)
    for addr, wgt in (
        (r0[perm], w00[perm]),
        (r0[perm] + 1, w01[perm]),
        (r0[perm] + W, w10[perm]),
        (r0[perm] + W + 1, w11[perm]),
    ):
        g = addr // P - _WBASE[blk]
        if g.min() < 0 or g.max() >= KWIN:
            raise ValueError("position fell outside its static 3-block window")
        np.add.at(st, (addr % P, (blk * KWIN + g) * P + lane), wgt)
    return perm, st.astype(np.float16)


def _build_program():
    import concourse.bacc as bacc
    import concourse.mybir as mybir
    from concourse.tile import TileContext

    f16 = mybir.dt.float16
    f32 = mybir.dt.float32

    nc = bacc.Bacc(
        "TRN2", target_bir_lowering=False, debug=False, num_devices=N_CORES
    )
    corr = nc.dram_tensor("corr", [HW, CH_HALF], f16, kind="ExternalInput").ap()
    st = nc.dram_tensor(
        "st", [P, N_BLOCKS * KWIN * P], f16, kind="ExternalInput"
    ).ap()
    out = nc.dram_tensor("out", [HW, CH_HALF], f16, kind="ExternalOutput").ap()

    with TileContext(nc) as tc:
        with (
            tc.tile_pool(name="stp", bufs=1) as stp,
            tc.tile_pool(name="cp", bufs=N_BLOCKS) as cp,
            tc.tile_pool(name="op", bufs=4) as op,
            tc.tile_pool(name="pp", bufs=2, space="PSUM") as pp,
        ):
            st_t = stp.tile([P, N_BLOCKS * KWIN * P], f16)
            nc.sync.dma_start(out=st_t[:], in_=st[:])

            c_tiles = []
            for j in range(N_BLOCKS):
                ct = cp.tile([P, CH_HALF], f16, tag="c")
                nc.sync.dma_start(out=ct[:], in_=corr[P * j : P * (j + 1), :])
                c_tiles.append(ct)

            for i in range(N_BLOCKS):
                w = int(_WBASE[i])
                ps = pp.tile([P, CH_HALF], f32, tag="ps")
                for g in range(KWIN):
                    lhsT = st_t[:, (i * KWIN + g) * P : (i * KWIN + g + 1) * P]
                    cj = c_tiles[w + g]
                    for c in range(0, CH_HALF, 512):
                        nc.tensor.matmul(
                            ps[:, c : c + 512],
                            lhsT,
                            cj[:, c : c + 512],
                            start=(g == 0),
                            stop=(g == KWIN - 1),
                        )
                ot = op.tile([P, CH_HALF], f16, tag="o")
                # alternate evacuation engines so neither becomes the
                # bottleneck behind the tensor engine
                if i % 2 == 0:
                    nc.vector.tensor_copy(ot[:], ps[:])
                else:
                    nc.scalar.copy(ot[:], ps[:])
                nc.gpsimd.dma_start(out=out[P * i : P * (i + 1), :], in_=ot[:])
    nc.compile()
    return nc


_cached = {}


def _get_program():
    if "nc" not in _cached:
        _cached["nc"] = _build_program()
    return _cached["nc"]


def _ensure_axon_hooks_importable():
    """bass_utils imports antenv.axon_hooks when tracing is requested (e.g.
    BASS_TRACE=1). Some containers ship an antenv stub without that module;
    provide a no-op registry so tracing degrades gracefully instead of
    crashing the run."""
    import sys
    import types

    try:
        import antenv.axon_hooks  # noqa: F401
    except Exception:
        m = types.ModuleType("antenv.axon_hooks")
        m._hook = None
        m.set_axon_ntff_profile_hook = lambda h: setattr(m, "_hook", h)
        m.get_axon_ntff_profile_hook = lambda: getattr(m, "_hook", None)
        sys.modules["antenv.axon_hooks"] = m


def kernel(correlation: np.ndarray, flow: np.ndarray, _trace: bool = False):
    _ensure_axon_hooks_importable()
    from concourse.bass_utils import run_bass_kernel_spmd

    correlation = np.ascontiguousarray(correlation, dtype=np.float32)
    flow = np.asarray(flow, dtype=np.float32)

    r0, w00, w01, w10, w11 = _host_indices_weights(flow)

    in_maps = []
    perms = []
    for b in range(B):
        perm, st = _batch_schedule(r0[b], w00[b], w01[b], w10[b], w11[b])
        perms.append(perm)
        corr_b = correlation[b].reshape(HW, HW).astype(np.float16)
        for half in range(2):
            in_maps.append(
                {
                    "corr": np.ascontiguousarray(
                        corr_b[:, half * CH_HALF : (half + 1) * CH_HALF]
                    ),
                    "st": st,
                }
            )

    nc = _get_program()
    extra = {"trace_cores": list(range(N_CORES))} if _trace else {}
    res = run_bass_kernel_spmd(
        nc, in_maps, core_ids=list(range(N_CORES)), trace=_trace, **extra
    )

    out = np.empty((B, HW, HW), dtype=np.float32)
    for core in range(N_CORES):
        b, half = divmod(core, 2)
        # device rows are in address-sorted order; scatter back to
        # natural position order
        out[b][perms[b], half * CH_HALF : (half + 1) * CH_HALF] = res.results[
            core
        ]["out"]
    if _trace:
        kernel.last_results = res
    return out.reshape(B, H, W, HW)


# revision 3
# speedup vs baseline: 3.9744x; 1.0571x over previous
"""CorrelationSampler Trainium2 kernel — banded-matmul formulation.

out[b, h, w, c] = bilinear sample of corr[b, :, :, c] at grid position
(h + flow_y, w + flow_x)-ish (align_corners=True, border padding).

Per batch this is out = S @ C where C = corr[b] viewed as a
[4096 src-position, 4096 channel] matrix and S is a [4096, 4096] sparse
matrix with the 4 bilinear corner weights per row. Sorting output
positions by their gather address makes S block-banded: each 128-row
out-block only touches src rows in a 3-aligned-block window around the
diagonal. The tensor engine then does the gather+blend as dense fp16
matmuls while C streams into SBUF exactly once — no 4x gather read
amplification, and fp16 halves all HBM traffic:

  - 8 cores = batch (4) x channel-half (2). Each core computes the full
    4096 sorted positions for its 2048-channel half of C.
  - Per out-block i (128 sorted positions): 3 window blocks x 4
    PSUM-bank matmuls of [128src,128pos]^T @ [128src,512ch], fp32 PSUM
    accumulate, then a PSUM->SBUF fp16 cast copy (alternating DVE /
    Activation engines) and a DMA store.
  - Host does the cheap parts: grid math, position sort, building the
    per-block stationary S^T tiles, fp16 conversion, and the final
    un-permute + fp32 upcast.
"""

import numpy as np

B, H, W = 4, 64, 64
HW = H * W  # 4096 source rows per batch; also 4096 output positions
CH_HALF = HW // 2  # 2048 channels per core
P = 128
N_BLOCKS = HW // P  # 32 out-blocks
KWIN = 3  # aligned src-blocks per out-block window
N_CORES = 8

# static window base per out-block: blocks {w_i, w_i+1, w_i+2}
_WBASE = np.minimum(np.maximum(np.arange(N_BLOCKS) - 1, 0), N_BLOCKS - KWIN)


def _host_indices_weights(flow: np.ndarray):
    """float32 replica of the reference's grid math -> corner row indices
    and bilinear corner weights, shape [B, H*W] each."""
    f32 = np.float32
    y_g, x_g = np.meshgrid(
        np.arange(H, dtype=f32), np.arange(W, dtype=f32), indexing="ij"
    )
    x_norm = (f32(2.0) * x_g / f32(W - 1) - f32(1.0)).astype(f32)
    y_norm = (f32(2.0) * y_g / f32(H - 1) - f32(1.0)).astype(f32)

    fx = flow[:, 0].astype(f32)
    fy = flow[:, 1].astype(f32)
    gx = x_norm[None] + fx / f32(W) * f32(2.0)
    gy = y_norm[None] + fy / f32(H) * f32(2.0)

    ix = np.clip((gx + f32(1.0)) * f32(0.5) * f32(W - 1), f32(0.0), f32(W - 1))
    iy = np.clip((gy + f32(1.0)) * f32(0.5) * f32(H - 1), f32(0.0), f32(H - 1))

    # floor is >= 0 after the clip; clamp to W-2/H-2 so the +1 neighbor
    # always exists. At the high border this gives weight 1.0 on the last
    # row/col -- identical result to the reference's clip formulation.
    ix0 = np.minimum(np.floor(ix), f32(W - 2)).astype(np.int32)
    iy0 = np.minimum(np.floor(iy), f32(H - 2)).astype(np.int32)
    wx = (ix - ix0.astype(f32)).astype(f32)
    wy = (iy - iy0.astype(f32)).astype(f32)

    one = f32(1.0)
    w00 = ((one - wy) * (one - wx)).astype(f32)
    w01 = ((one - wy) * wx).astype(f32)
    w10 = (wy * (one - wx)).astype(f32)
    w11 = (wy * wx).astype(f32)

    r0 = (iy0 * np.int32(W) + ix0).reshape(B, HW)
    flat = lambda a: a.reshape(B, HW)
    return r0, flat(w00), flat(w01), flat(w10), flat(w11)


def _batch_schedule(r0, w00, w01, w10, w11):
    """For one batch: sort positions by gather address, build the
    stationary S^T tiles for the static 3-block windows.

    Returns (perm [HW], st [P, N_BLOCKS*KWIN*P] fp16). Row rk of the
    device output corresponds to original position perm[rk]."""
    perm = np.argsort(r0, kind="stable")
    rk = np.arange(HW)
    blk = rk // P
    lane = rk % P
    st = np.zeros((P, N_BLOCKS * KWIN * P), np.float32)
    for addr, wgt in (
        (r0[perm], w00[perm]),
        (r0[perm] + 1, w01[perm]),
        (r0[perm] + W, w10[perm]),
        (r0[perm] + W + 1, w11[perm]),
    ):
        g = addr // P - _WBASE[blk]
        if g.min() < 0 or g.max() >= KWIN:
            raise ValueError("position fell outside its static 3-block window")
        np.add.at(st, (addr % P, (blk * KWIN + g) * P + lane), wgt)
    return perm, st.astype(np.float16)


def _build_program():
    import concourse.bacc as bacc
    import concourse.mybir as mybir
    from concourse.tile import TileContext

    f16 = mybir.dt.float16
    f32 = mybir.dt.float32

    nc = bacc.Bacc(
        "TRN2", target_bir_lowering=False, debug=False, num_devices=N_CORES
    )
    corr = nc.dram_tensor("corr", [HW, CH_HALF], f16, kind="ExternalInput").ap()
    st = nc.dram_tensor(
        "st", [P, N_BLOCKS * KWIN * P], f16, kind="ExternalInput"
    ).ap()
    out = nc.dram_tensor("out", [HW, CH_HALF], f16, kind="ExternalOutput").ap()

    # S is loaded in 4 chunks of 8 out-blocks interleaved with the C
    # stream so the first matmul starts after ~2MB of DMA, not after the
    # whole 3MB S tensor. All loads share the sync queue => the DMA
    # device executes them in issue order.
    SCHUNK = 8  # out-blocks per S chunk
    SCW = SCHUNK * KWIN * P  # S columns per chunk

    with TileContext(nc) as tc:
        with (
            tc.tile_pool(name="stp", bufs=4) as stp,
            tc.tile_pool(name="cp", bufs=N_BLOCKS) as cp,
            tc.tile_pool(name="op", bufs=4) as op,
            tc.tile_pool(name="pp", bufs=8, space="PSUM") as pp,
        ):
            st_chunks = []
            c_tiles = []

            def load_c(j):
                ct = cp.tile([P, CH_HALF], f16, tag="c")
                nc.sync.dma_start(out=ct[:], in_=corr[P * j : P * (j + 1), :])
                c_tiles.append(ct)

            def load_s(k):
                sct = stp.tile([P, SCW], f16, tag="s")
                nc.sync.dma_start(out=sct[:], in_=st[:, k * SCW : (k + 1) * SCW])
                st_chunks.append(sct)

            load_s(0)
            for j in range(3):
                load_c(j)
            load_s(1)
            for j in range(3, 9):
                load_c(j)
            load_s(2)
            for j in range(9, 17):
                load_c(j)
            load_s(3)
            for j in range(17, N_BLOCKS):
                load_c(j)

            for i in range(N_BLOCKS):
                w = int(_WBASE[i])
                sch = st_chunks[i // SCHUNK]
                scol = (i % SCHUNK) * KWIN * P
                ot = op.tile([P, CH_HALF], f16, tag="o")
                # bank-outer / window-inner: each 512-wide PSUM bank
                # finishes its accumulation group early and is evacuated
                # (and freed for block i+2) while the tensor engine moves
                # on to the next bank
                for c in range(4):
                    ps = pp.tile([P, 512], f32, tag="ps")
                    for g in range(KWIN):
                        nc.tensor.matmul(
                            ps[:],
                            sch[:, scol + g * P : scol + (g + 1) * P],
                            c_tiles[w + g][:, 512 * c : 512 * (c + 1)],
                            start=(g == 0),
                            stop=(g == KWIN - 1),
                        )
                    if c % 2 == 0:
                        nc.vector.tensor_copy(ot[:, 512 * c : 512 * (c + 1)], ps[:])
                    else:
                        nc.scalar.copy(ot[:, 512 * c : 512 * (c + 1)], ps[:])
                nc.gpsimd.dma_start(out=out[P * i : P * (i + 1), :], in_=ot[:])
    nc.compile()
    return nc


_cached = {}


def _get_program():
    if "nc" not in _cached:
        _cached["nc"] = _build_program()
    return _cached["nc"]


def _ensure_axon_hooks_importable():
    """bass_utils imports antenv.axon_hooks when tracing is requested (e.g.
    BASS_TRACE=1). Some containers ship an antenv stub without that module;
    provide a no-op registry so tracing degrades gracefully instead of
    crashing the run."""
    import sys
    import types

    try:
        import antenv.axon_hooks  # noqa: F401
    except Exception:
        m = types.ModuleType("antenv.axon_hooks")
        m._hook = None
        m.set_axon_ntff_profile_hook = lambda h: setattr(m, "_hook", h)
        m.get_axon_ntff_profile_hook = lambda: getattr(m, "_hook", None)
        sys.modules["antenv.axon_hooks"] = m


def kernel(correlation: np.ndarray, flow: np.ndarray, _trace: bool = False):
    _ensure_axon_hooks_importable()
    from concourse.bass_utils import run_bass_kernel_spmd

    correlation = np.ascontiguousarray(correlation, dtype=np.float32)
    flow = np.asarray(flow, dtype=np.float32)

    r0, w00, w01, w10, w11 = _host_indices_weights(flow)

    in_maps = []
    perms = []
    for b in range(B):
        perm, st = _batch_schedule(r0[b], w00[b], w01[b], w10[b], w11[b])
        perms.append(perm)
        corr_b = correlation[b].reshape(HW, HW).astype(np.float16)
        for half in range(2):
            in_maps.append(
                {
                    "corr": np.ascontiguousarray(
                        corr_b[:, half * CH_HALF : (half + 1) * CH_HALF]
                    ),
                    "st": st,
                }
            )

    nc = _get_program()
    extra = {"trace_cores": list(range(N_CORES))} if _trace else {}
    res = run_bass_kernel_spmd(
        nc, in_maps, core_ids=list(range(N_CORES)), trace=_trace, **extra
    )

    out = np.empty((B, HW, HW), dtype=np.float32)
    for core in range(N_CORES):
        b, half = divmod(core, 2)
        # device rows are in address-sorted order; scatter back to
        # natural position order
        out[b][perms[b], half * CH_HALF : (half + 1) * CH_HALF] = res.results[
            core
        ]["out"]
    if _trace:
        kernel.last_results = res
    return out.reshape(B, H, W, HW)
